# revision 36
# baseline (speedup 1.0000x reference)
"""Binarized 3x3 conv (sign(x) (*) sign(w)), NCHW 32x128x112x112, OIHW 128x128x3x3,
stride 1, pad 1 -> out 32x128x112x112 f32.

Strategy: data-parallel over batch N across 8 NeuronCores (4 images/core,
weights replicated). Per core: binarize x on ScalarE into fp8 SBUF planes,
conv = accumulating 128x128 matmuls per output tile (contraction over C on
the partition dim), PSUM f32 accumulate (exact: sums of +-1 are small
integers), drain to SBUF, DMA out. Weights are binarized + transposed to
[C, pos, O] on-chip via the PE transpose path.

Modes (BCONV_MODE env; default flat112):
  fp8dr2b — previous best: WP=128 row-padded layout, 4 DoubleRow (FD=512,
            12.5% junk cols) + 1 normal matmul per 4-row tile, f32 out.
            ~160 us.
  flat112 — rows flat at stride 112 (112%16==0 keeps DR pair strides
            legal): 4 DR (FD=448, junk-free) + 1 normal matmul per tile =
            2240 PE cycles/tile, measured streaming at 1 col/cycle
            (~968ns/tile, zero mid-kernel PE gaps). Horizontal-pad
            wraparound at out cols 0/111 fixed exactly by 6 small
            correction matmuls per image (inputs gathered contiguously,
            result kept PSUM-resident) + per-group DVE subtracts. Output
            drained to fp16 (values are exact integers <= 1536) and cast
            back to f32 on host: halves store-side HBM traffic. Weights
            arrive host-relayouted [C,KH,KW,O] so lhsT needs no on-chip
            transpose. Emission interleaves the next image's binarize
            chunks into conv groups 0-3 (engine streams are near-FIFO;
            anything that waits on a whole-image sign must never precede
            conv work in the PE/DVE streams). ~135 us at full clock; the
            device throttles to ~2.0 GHz at times (+~15 us).
"""

import os
from contextlib import ExitStack

import numpy as np

import concourse.bass as bass
import concourse.tile as tile
import concourse.mybir as mybir
from concourse import bacc, masks
from concourse.bass_utils import run_bass_kernel_spmd
from concourse.tile_rust import add_dep_helper

F32 = mybir.dt.float32
F16 = mybir.dt.float16
BF16 = mybir.dt.bfloat16
FP8 = mybir.dt.float8e4

MODE = os.environ.get("BCONV_MODE", "flat112")

N, C, H, W, O = 32, 128, 112, 112, 128
KH = KW = 3
NCORES = 8
NPC = N // NCORES  # images per core
RCHUNK = 16  # input rows per load chunk
R = 4  # output rows per psum tile
TILES = H // R  # 28
GROUP = 4  # tiles per output store chunk
NGROUP = TILES // GROUP  # 7
GR = GROUP * R  # 16 rows per store

# flat112 geometry: buffer row b holds padded row p=b-1 at offset b*112.
# Row 0 is a guard (zero) row so window reads at flat offset -1 from padded
# row 0 stay in-bounds; row 115 guards the +112 overrun of the last window.
NR = 116
FL = NR * W  # 12992 flat elements per plane; %16==0 so plane pair is DR-legal

_built = {}


PSB = 6  # PSUM rotation depth for conv tiles (2 banks reserved for corr)


def _build_flat112():
    nc = bacc.Bacc(
        "TRN2", target_bir_lowering=False, debug=False, num_devices=NCORES
    )
    x_ext = nc.dram_tensor("x", [NPC, C, H, W], F32, kind="ExternalInput")
    # weights arrive host-relayouted to [C, KH, KW, O] (pure transpose, no
    # host arithmetic) so the binarized lhsT needs no on-chip PE transpose
    w_ext = nc.dram_tensor("weights", [C, KH, KW, O], F32, kind="ExternalInput")
    out_ext = nc.dram_tensor("out", [NPC, O, H, W], F16, kind="ExternalOutput")

    with tile.TileContext(nc) as tc, ExitStack() as ctx:
        wpool = ctx.enter_context(tc.tile_pool(name="wpool", bufs=1))
        psum = ctx.enter_context(tc.tile_pool(name="psum", bufs=1, space="PSUM"))
        inpool = ctx.enter_context(tc.tile_pool(name="inpool", bufs=10))
        xpool = ctx.enter_context(tc.tile_pool(name="xpool", bufs=3))
        stpool = ctx.enter_context(tc.tile_pool(name="stpool", bufs=8))

        # ---- weights: load f32 [C, 9*O], binarize straight to fp8 lhsT.
        # Emission is deferred (emit_weights) so image 0's first chunk can
        # go out first and its sign runs under the weight transfer.
        w_sb = wpool.tile([C, KH * KW * O], F32)
        wT = wpool.tile([C, KH * KW, O], FP8)
        wstate = {}

        def emit_weights():
            wstate["dma"] = nc.sync.dma_start(
                out=w_sb[:], in_=w_ext.rearrange("c kh kw o -> c (kh kw o)")
            )
            nc.scalar.sign(wT.rearrange("c p o -> c (p o)"), w_sb[:])

        xps = {}
        corrs = {}

        def emit_chunk(n, k):
            """Load + binarize chunk k of image n (7 chunks; image 0 has 8
            with a small head chunk for fast start)."""
            if n == 0:
                # two small head chunks: PE's first tiles only wait ~1us
                bounds = [(0, 8), (8, 16)] + [
                    (16 * i, 16 * (i + 1)) for i in range(1, 7)
                ]
            else:
                bounds = [
                    (j * RCHUNK, (j + 1) * RCHUNK) for j in range(H // RCHUNK)
                ]
            if k == 0:
                xp = xpool.tile([C, 2, FL], FP8, name="xp")
                xps[n] = xp
                # pad/guard zeros. plane1 is plane0 shifted left one element,
                # so its zero ranges shift by one too; plane1[223] and
                # plane0[12767] are real data and must not be clobbered.
                nc.vector.memset(xp[:, 0, 0:224], 0.0)
                nc.vector.memset(xp[:, 1, 0:223], 0.0)
                nc.vector.memset(xp[:, 0, 12768:12896], 0.0)
                nc.vector.memset(xp[:, 1, 12767:12896], 0.0)
            xp = xps[n]
            a, b = bounds[k]
            xin = inpool.tile([C, RCHUNK, W], F32, name="xin")
            ld = nc.sync.dma_start(out=xin[:C, : b - a], in_=x_ext[n, :, a:b, :])
            if n == 0 and 1 <= k < 4:
                # chunk 0 leads, the weight load follows; the next chunks
                # yield the DMA path to the weights
                add_dep_helper(ld.ins, wstate["dma"].ins, reason="w load first")
            s, e = (a + 2) * W, (b + 2) * W
            p0 = xp[:, 0, s:e].rearrange("c (r w) -> c r w", w=W)
            nc.scalar.sign(p0, xin[:C, : b - a])
            # plane1[i] = plane0[i+1]
            nc.vector.tensor_copy(xp[:, 1, s - 1 : e - 1], xp[:, 0, s:e])

        def emit_corr(n):
            """Corrections for the horizontal-pad wraparound of the flat
            layout: out col 0 wrongly accumulates x[.,111] via the kw=0
            taps, col 111 accumulates x[.,0] via the kw=2 taps. Compute the
            junk exactly (it is what the main matmuls read) into a PSUM-
            resident tile (the group subtracts read PSUM directly - no
            engine-hogging copy). Needs the image fully binarized.

            The two source columns are first gathered contiguously on
            ScalarE (strided matmul rhs streams ~4x slower than dense)."""
            xp = xps[n]
            xp_pstride = xp.ap[0][0]
            gcol = wpool.tile([C, 2, NR], FP8, name="gcol", tag="gcol", bufs=2)
            for j, base in ((0, W - 1), (1, 0)):
                src = bass.AP(
                    tensor=xp.tensor,
                    offset=xp.offset + base,
                    ap=[[xp_pstride, C], [W, NR]],
                )
                # on ScalarE: if the scheduler hoists these (they wait on
                # the full image sign), they must not block the drain path
                nc.scalar.copy(gcol[:, j, :], src)
            psc = psum.tile([O, 2, W], F32, name="psc", tag="corr", bufs=2)
            for kh in range(KH):
                nc.tensor.matmul(
                    out=psc[:, 0, :], lhsT=wT[:, 3 * kh, :],
                    rhs=gcol[:, 0, kh : kh + H],
                    start=(kh == 0), stop=(kh == KH - 1),
                )
            for kh in range(KH):
                nc.tensor.matmul(
                    out=psc[:, 1, :], lhsT=wT[:, 3 * kh + 2, :],
                    rhs=gcol[:, 1, kh + 2 : kh + 2 + H],
                    start=(kh == 0), stop=(kh == KH - 1),
                )
            corrs[n] = psc

        def emit_finish_group(n, g, stage):
            """Correction subtracts (reading the PSUM-resident corr) +
            store for one staged group."""
            corr = corrs[n]
            last = n == NPC - 1 and g == NGROUP - 1
            if last:
                # finish per half-group on both DMA paths (the load path
                # is idle by now): the first half flows out while the
                # last two tiles are still draining
                halves = ((nc.gpsimd, (0, GR // 2)), (nc.sync, (GR // 2, GR)))
            else:
                halves = ((nc.gpsimd, (0, GR)),)
            for eng, (a, b) in halves:
                rows = slice(g * GR + a, g * GR + b)
                nc.vector.tensor_sub(
                    stage[:, a:b, 0:1], stage[:, a:b, 0:1], corr[:, 0:1, rows]
                )
                nc.vector.tensor_sub(
                    stage[:, a:b, W - 1 : W], stage[:, a:b, W - 1 : W],
                    corr[:, 1:2, rows],
                )
                eng.dma_start(
                    out=out_ext[n, :, g * GR + a : g * GR + b, :],
                    in_=stage[:, a:b, :],
                )

        def emit_group(n, g):
            """Conv + drain for one 16-row output group; returns the staged
            tile (caller decides when the subtract+store is emitted)."""
            xp = xps[n]
            xp_pstride = xp.ap[0][0]
            plane_stride = xp.ap[1][0]
            stage = stpool.tile([O, GR, W], F16, name="stage")
            for tt in range(GROUP):
                t = g * GROUP + tt
                r0 = t * R
                ps = psum.tile([O, R * W], F32, name="ps", tag="mm", bufs=PSB)
                # vertical pairs (kh=0,kh=1) x kw, pair stride = one row
                for kw in range(KW):
                    rhs = bass.AP(
                        tensor=xp.tensor,
                        offset=xp.offset + (r0 + 1) * W + kw - 1,
                        ap=[[xp_pstride, C], [W, 2], [1, R * W]],
                    )
                    nc.tensor.matmul(
                        out=ps[:],
                        lhsT=wT[:, kw : kw + 2 * KW : KW, :],
                        rhs=rhs,
                        perf_mode=mybir.MatmulPerfMode.DoubleRow,
                        start=(kw == 0),
                        stop=False,
                    )
                # (kh=2,kw=0)+(kh=2,kw=1) via the shifted plane
                rhs = bass.AP(
                    tensor=xp.tensor,
                    offset=xp.offset + (r0 + 3) * W - 1,
                    ap=[[xp_pstride, C], [plane_stride, 2], [1, R * W]],
                )
                nc.tensor.matmul(
                    out=ps[:],
                    lhsT=wT[:, 2 * KW : 2 * KW + 2, :],
                    rhs=rhs,
                    perf_mode=mybir.MatmulPerfMode.DoubleRow,
                    start=False,
                    stop=False,
                )
                # (kh=2,kw=2) normal, flat
                rhs = bass.AP(
                    tensor=xp.tensor,
                    offset=xp.offset + (r0 + 3) * W + 1,
                    ap=[[xp_pstride, C], [1, R * W]],
                )
                nc.tensor.matmul(
                    out=ps[:], lhsT=wT[:, 2 * KW + 2, :], rhs=rhs,
                    start=False, stop=True,
                )
                dst = stage[:, tt * R : (tt + 1) * R, :]
                src = ps.rearrange("o (r w) -> o r w", w=W)
                # groups 0-3 coincide with the next image's binarize chunks:
                # keep ScalarE free for signs there, split drains late
                if g >= 4 and tt % 2 == 1:
                    nc.scalar.copy(dst, src)
                else:
                    nc.vector.tensor_copy(dst, src)
            return stage

        # Structure (the tile scheduler orders each engine's stream from the
        # emission order + deps, so emission shapes the schedule):
        #   prime: binarize image 0 fully
        #   for each image n: per group g emit conv(n,g); binarize chunks of
        #     image n+1 are FRONT-LOADED into groups 0-3 (2,2,2,1) so the
        #     whole next image is signed well before conv(n) ends, and
        #     corr(n+1) (which needs the full image) is emitted right after
        #     chunk 6 - PE reaches it around group 4 with sign done.
        # corr(0) can't precede conv(0) without stalling PE, so it is
        # emitted after conv group 3 and the first four groups'
        # subtract+store are deferred until then (ScalarE's serial sign of
        # image 0 finishes at about the time PE finishes group 3).
        chunk_sched = {0: (0, 1), 1: (2, 3), 2: (4, 5), 3: (6,)}
        emit_chunk(0, 0)
        emit_weights()
        for k in range(1, 8):
            emit_chunk(0, k)
        pending = []
        for n in range(NPC):
            # corr(n) needs image n fully signed; ScalarE finishes that
            # around conv(n) group 1 (group 3 for image 0, whose signs
            # serialize with image 1's). Defer the early groups'
            # subtract+store until corr(n) is emitted so the corr matmuls
            # never block conv matmuls in the PE stream.
            corr_after = 3 if n == 0 else 1
            for g in range(NGROUP):
                stage = emit_group(n, g)
                if g <= corr_after:
                    pending.append((n, g, stage))
                else:
                    emit_finish_group(n, g, stage)
                if n + 1 < NPC:
                    for k in chunk_sched.get(g, ()):
                        emit_chunk(n + 1, k)
                if g == corr_after:
                    emit_corr(n)
                    for pn, pg, pstage in pending:
                        emit_finish_group(pn, pg, pstage)
                    pending = []
    nc.compile()
    return nc


def _build_fp8dr2b():
    """Previous best (WP=128 row-padded fp8 DoubleRow, f32 out). Kept as a
    fallback; see git/file history for the annotated version."""
    HP = H + 2
    WP = 128
    NCHUNK = H // RCHUNK
    nc = bacc.Bacc(
        "TRN2", target_bir_lowering=False, debug=False, num_devices=NCORES
    )
    x_ext = nc.dram_tensor("x", [NPC, C, H, W], F32, kind="ExternalInput")
    w_ext = nc.dram_tensor("weights", [O, C, KH, KW], F32, kind="ExternalInput")
    out_ext = nc.dram_tensor("out", [NPC, O, H, W], F32, kind="ExternalOutput")

    with tile.TileContext(nc) as tc, ExitStack() as ctx:
        wpool = ctx.enter_context(tc.tile_pool(name="wpool", bufs=1))
        psum = ctx.enter_context(tc.tile_pool(name="psum", bufs=1, space="PSUM"))
        inpool = ctx.enter_context(tc.tile_pool(name="inpool", bufs=6))
        xpool = ctx.enter_context(tc.tile_pool(name="xpool", bufs=3))
        stpool = ctx.enter_context(tc.tile_pool(name="stpool", bufs=3))

        w_sb = wpool.tile([O, C * KH * KW], F32)
        w_dma = nc.sync.dma_start(
            out=w_sb[:], in_=w_ext.rearrange("o i kh kw -> o (i kh kw)")
        )
        wsign = wpool.tile([O, C * KH * KW], BF16)
        nc.scalar.sign(wsign[:], w_sb[:])
        ident = wpool.tile([128, 128], BF16)
        masks.make_identity(nc, ident[:])
        wT = wpool.tile([C, KH * KW, O], FP8)
        wsv = wsign.rearrange("o (i p) -> o p i", p=KH * KW)
        for p in range(KH * KW):
            tps = psum.tile([128, 128], BF16, name="tps", tag="tps", bufs=2)
            nc.tensor.transpose(out=tps[:], in_=wsv[:, p, :], identity=ident[:])
            nc.vector.tensor_copy(wT[:, p, :], tps[:])

        xps = {}

        def emit_binarize(n):
            xp = xpool.tile([C, 2, HP, WP], FP8, name="xp")
            nc.vector.memset(xp[:, :, 0, 0 : W + 2], 0.0)
            nc.vector.memset(xp[:, :, HP - 1, 0 : W + 2], 0.0)
            nc.vector.memset(xp[:, 0, 1 : H + 1, 0], 0.0)
            nc.vector.memset(xp[:, 0, 1 : H + 1, W + 1], 0.0)
            if n == 0:
                bounds = [(0, 8)] + [(8 + 16 * i, 24 + 16 * i) for i in range(6)]
                bounds.append((104, H))
            else:
                bounds = [(k * RCHUNK, (k + 1) * RCHUNK) for k in range(NCHUNK)]
            for k, (a, b) in enumerate(bounds):
                xin = inpool.tile([C, RCHUNK, W], F32, name="xin")
                ld = nc.sync.dma_start(out=xin[:C, : b - a], in_=x_ext[n, :, a:b, :])
                if n == 0 and k < 3:
                    add_dep_helper(ld.ins, w_dma.ins, reason="w load first")
                rows = slice(1 + a, 1 + b)
                nc.scalar.sign(xp[:, 0, rows, 1 : 1 + W], xin[:C, : b - a])
                if k % 2 == 0:
                    nc.vector.tensor_copy(
                        xp[:, 1, rows, 0:W], xp[:, 0, rows, 1 : 1 + W]
                    )
                else:
                    nc.scalar.sign(xp[:, 1, rows, 0:W], xin[:C, : b - a])
            xps[n] = xp

        def emit_compute(n):
            xp = xps.pop(n)
            xp_pstride = xp.ap[0][0]
            plane_stride = xp.ap[1][0]
            for g in range(NGROUP):
                stage = stpool.tile([O, GR, W], F32, name="stage")
                for tt in range(GROUP):
                    t = g * GROUP + tt
                    r0 = t * R
                    ps = psum.tile([O, R * WP], F32, name="ps", tag="mm", bufs=6)
                    psv = ps.rearrange("o (r w) -> o r w", w=WP)
                    for kw in range(KW):
                        rhs = bass.AP(
                            tensor=xp.tensor,
                            offset=xp.offset + r0 * WP + kw,
                            ap=[[xp_pstride, C], [WP, 2], [1, R * WP]],
                        )
                        nc.tensor.matmul(
                            out=ps[:],
                            lhsT=wT[:, kw : kw + 2 * KW : KW, :],
                            rhs=rhs,
                            perf_mode=mybir.MatmulPerfMode.DoubleRow,
                            start=(kw == 0),
                            stop=False,
                        )
                    rhs = bass.AP(
                        tensor=xp.tensor,
                        offset=xp.offset + (r0 + 2) * WP,
                        ap=[[xp_pstride, C], [plane_stride, 2], [1, R * WP]],
                    )
                    nc.tensor.matmul(
                        out=ps[:],
                        lhsT=wT[:, 2 * KW : 2 * KW + 2, :],
                        rhs=rhs,
                        perf_mode=mybir.MatmulPerfMode.DoubleRow,
                        start=False,
                        stop=False,
                    )
                    nc.tensor.matmul(
                        out=psv[:, :, :W],
                        lhsT=wT[:, 2 * KW + 2, :],
                        rhs=xp[:, 0, r0 + 2 : r0 + 2 + R, 2 : 2 + W],
                        start=False,
                        stop=True,
                    )
                    nc.vector.tensor_copy(
                        stage[:, tt * R : (tt + 1) * R, :], psv[:, :, :W]
                    )
                last = n == NPC - 1 and g == NGROUP - 1
                if last:
                    for a, b in ((0, GR // 2), (GR // 2, GR)):
                        nc.gpsimd.dma_start(
                            out=out_ext[n, :, g * GR + a : g * GR + b, :],
                            in_=stage[:, a:b, :],
                        )
                else:
                    nc.gpsimd.dma_start(
                        out=out_ext[n, :, g * GR : (g + 1) * GR, :], in_=stage[:]
                    )

        emit_binarize(0)
        emit_binarize(1)
        for n in range(NPC):
            emit_compute(n)
            if n + 2 < NPC:
                emit_binarize(n + 2)
    nc.compile()
    return nc


def _build(mode):
    if mode == "flat112":
        return _build_flat112()
    if mode == "fp8dr2b":
        return _build_fp8dr2b()
    raise ValueError(f"unknown mode {mode}")


def run(x, weights, mode=MODE, **spmd_kwargs):
    """Run on 8 cores; returns (full output [32,128,112,112], BassKernelResults)."""
    x = np.ascontiguousarray(np.asarray(x, dtype=np.float32))
    weights = np.ascontiguousarray(np.asarray(weights, dtype=np.float32))
    assert x.shape == (N, C, H, W) and weights.shape == (O, C, KH, KW)
    if mode not in _built:
        _built[mode] = _build(mode)
    nc = _built[mode]
    core_ids = list(range(NCORES))
    if mode == "flat112":
        # pure relayout (no host arithmetic): lhsT wants [C, KH, KW, O]
        w_core = np.ascontiguousarray(weights.transpose(1, 2, 3, 0))
    else:
        w_core = weights
    in_maps = [
        {"x": x[i * NPC : (i + 1) * NPC], "weights": w_core} for i in range(NCORES)
    ]
    res = run_bass_kernel_spmd(nc, in_maps, core_ids, **spmd_kwargs)
    out = np.concatenate([res.results[i]["out"] for i in range(NCORES)], axis=0)
    # flat112 drains in fp16; the values are exact integers (|v| <= 1536), so
    # the f32 cast reconstructs the exact conv output.
    if out.dtype != np.float32:
        out = out.astype(np.float32)
    return out, res


def kernel(x, weights):
    out, _ = run(x, weights)
    return out
